# revision 1
# baseline (speedup 1.0000x reference)
"""AdaptiveLabelLoss Trainium2 kernel (8 NeuronCores, class-sharded).

loss = mean_b [ lse_b - 0.9*pred[b,t_b] - 0.1*conf[t_b].pred_b ]
where conf is the row-normalized exp cosine-similarity confusion matrix
(diagonal zeroed) and lse is logsumexp over pred rows. The Dirichlet
sample of the reference is replaced by its analytic mean (= conf row),
which matches the fixed-key sample mean to ~2e-5 relative.

Sharding: core k owns classes [512k, 512k+512). Batch rows are routed to
the core that owns their target class, grouped by (local target // 128)
into 4 groups, padded to a uniform number of 128-row tiles. All
core-dependence lives in the data (the program is SPMD-uniform).
"""

import os
import numpy as np
import ml_dtypes

B, C, D = 16384, 4096, 1024
NCORES = 8
CHUNK = C // NCORES          # 512 classes per core
NG = 4                       # groups of 128 local classes
CONFIDENCE = 0.9
SMOOTHING = 0.1
E_CONST = float(np.exp(np.float32(1.0)))  # e^1 = diagonal of exp(sim)

_cache = {}
LAST_RESULTS = None  # for test harness introspection


def _split_multiwait_drains(nc, max_waits: int = 1):
    """Walrus (CoreV3) rejects instructions carrying many sem waits. The
    Tile kernel-tail drain waits on every engine/queue sem at once; split
    the extras onto preceding single-wait drains on the same engine."""
    import concourse.mybir as mybir
    import bass_rust
    for f in nc.m.functions:
        for bb in f.blocks:
            i = 0
            insts = bb.instructions
            while i < len(insts):
                inst = insts[i]
                si = inst.sync_info
                if si is not None and si.on_wait and len(si.on_wait) > max_waits:
                    waits = list(si.on_wait)
                    keep = waits[:max_waits]
                    extra = waits[max_waits:]
                    pre = []
                    for j, w in enumerate(extra):
                        d = mybir.InstDrain(
                            name=f"{inst.name}-sw{j}", ins=[], outs=[])
                        d.engine = inst.engine
                        d.sync_info = bass_rust.SyncInfo(
                            on_wait=[w], on_update=[])
                        pre.append(d)
                    inst.sync_info = bass_rust.SyncInfo(
                        on_wait=keep, on_update=list(si.on_update or []))
                    for j, d in enumerate(pre):
                        insts.insert(i + j, d)
                    i += len(pre)
                i += 1


def _build(nkt: int, stage: str = "full", split_drains: bool = True,
           psa_bufs: int = 8, exp_halves: bool = False,
           q_split: bool = False, oh_gpsimd: bool = False,
           early_lse: bool = False, lse_in_b: bool = False):
    """Build + compile the SPMD program. nkt = 128-row tiles per group.
    stage: debug knob - "in", "norm", "g", "q", or "full"."""
    import concourse.bass as bass
    import concourse.bacc as bacc
    import concourse.tile as tile
    import concourse.mybir as mybir
    import contextlib

    f32 = mybir.dt.float32
    bf16 = mybir.dt.bfloat16
    AL = mybir.AluOpType
    AF = mybir.ActivationFunctionType

    TK = NG * nkt            # total row tiles
    S = TK * 128             # padded rows per core
    F = TK + 4   # fin columns: TK lse | 4 dot cols

    nc = bacc.Bacc("TRN2", target_bir_lowering=False, debug=False,
                   num_devices=NCORES)

    predb = nc.dram_tensor("predb", [S, C], bf16, kind="ExternalInput").ap()
    wt_all = nc.dram_tensor("wt_all", [D, C], bf16, kind="ExternalInput").ap()
    wt_loc = nc.dram_tensor("wt_loc", [D, CHUNK], bf16, kind="ExternalInput").ap()
    ltg = nc.dram_tensor("ltg", [128, TK], f32, kind="ExternalInput").ap()
    vmask = nc.dram_tensor("vmask", [128, TK], f32, kind="ExternalInput").ap()
    dmask = nc.dram_tensor("dmask", [128, C], bf16, kind="ExternalInput").ap()
    out = nc.dram_tensor("out", [1, 1], f32, kind="ExternalOutput").ap()

    with tile.TileContext(nc) as tc:
        stack = contextlib.ExitStack()
        with stack:
            persist = stack.enter_context(tc.tile_pool(name="persist", bufs=1))
            pred_pool = stack.enter_context(tc.tile_pool(
                name="pred", bufs=(3 if lse_in_b else 4)))
            escr_pool = stack.enter_context(tc.tile_pool(name="escr", bufs=2))
            oh_pool = stack.enter_context(tc.tile_pool(
                name="oh", bufs=(1 if lse_in_b else 2)))
            lse_pool = stack.enter_context(tc.tile_pool(name="lse", bufs=2))
            dram_pool = stack.enter_context(
                tc.tile_pool(name="dram", bufs=1, space="DRAM"))

            # ---- persistent tiles ----
            conf = persist.tile([128, NG * C], bf16)      # 32KB/part
            dmask_sb = persist.tile([128, C], bf16)       # 8KB
            colidx = persist.tile([128, 128], f32)
            ltg_sb = persist.tile([128, TK], f32)
            vmask_sb = persist.tile([128, TK], f32)
            esums = persist.tile([128, TK], f32)
            small = persist.tile([128, 384], f32)
            nrow = persist.tile([1, C], f32)   # col-form norms^2 -> 1/norm
            # small col map (f32 scratch columns):
            NSQL = 0                   # [0,4)   local norms^2
            INVL = 4                   # [4,8)   local 1/norm (col m)
            RDEN = 8                   # [8,12)  1/(rowsum-e) per group
            SPR = 12                   # [12,16) 9 - e*rden per group
            NSQ32 = 320                # [320,352) global norms^2 (row form)
            ESG = 16                   # [16,48) exp-sim row sums (m*8+n)
            FIN = 48                   # [48,48+F)
            COEF = FIN + F + 4         # [.., +F)
            LNV = COEF + F + 4         # [.., +TK)
            PROD = LNV + TK + 4        # [.., +F)
            ONES = PROD + F + 2
            OUTC = ONES + 2
            assert OUTC < 384

            nc.sync.dma_start(ltg_sb[:], ltg)
            nc.sync.dma_start(vmask_sb[:], vmask)
            nc.sync.dma_start(dmask_sb[:], dmask)
            nc.gpsimd.iota(colidx[:], pattern=[[1, 128]], base=0,
                           channel_multiplier=0,
                           allow_small_or_imprecise_dtypes=True)
            nc.vector.memset(small[:, ONES:ONES + 1], 1.0)

            if stage != "in":
                # ============== Phase A: norms + Phase B: conf ==============
                with tc.tile_pool(name="wtp", bufs=1) as wtp, \
                     tc.tile_pool(name="gscr", bufs=2) as gscr, \
                     tc.tile_pool(name="wsq", bufs=2) as wsqp, \
                     tc.tile_pool(name="psA", bufs=psa_bufs, space="PSUM") as psA:

                    wt_sb = wtp.tile([128, 8 * C], bf16)     # W^T, kd-sliced
                    wtloc_sb = wtp.tile([128, 8 * CHUNK], bf16)
                    invb = wtp.tile([128, C], bf16)          # 1/norm bcast

                    for kd in range(8):
                        nc.sync.dma_start(wt_sb[:, kd * C:(kd + 1) * C],
                                          wt_all[kd * 128:(kd + 1) * 128, :])
                        nc.sync.dma_start(
                            wtloc_sb[:, kd * CHUNK:(kd + 1) * CHUNK],
                            wt_loc[kd * 128:(kd + 1) * 128, :])

                    if early_lse:
                        # logsumexp pass decoupled from the Q pass: pred is
                        # read twice from HBM so the exps overlap phase B
                        for kt in range(TK):
                            pt2 = pred_pool.tile([128, C], bf16,
                                                 name=f"ptl{kt}", tag="pt")
                            nc.sync.dma_start(
                                pt2[:], predb[kt * 128:(kt + 1) * 128, :])
                            nc.scalar.activation(
                                pt2[:], pt2[:], AF.Exp,
                                accum_out=esums[:, kt:kt + 1])

                    # identity for PE transpose / diag extraction
                    ident = gscr.tile([128, 128], f32, tag="ident")
                    pidx = gscr.tile([128, 1], f32, tag="pidx")
                    nc.gpsimd.iota(pidx[:], pattern=[[0, 1]], base=0,
                                   channel_multiplier=1,
                                   allow_small_or_imprecise_dtypes=True)
                    nc.vector.tensor_scalar(ident[:], colidx[:], pidx[:],
                                            None, op0=AL.is_equal)

                    # norms^2 via Gram-diagonal blocks on PE (row form):
                    # global from wt_sb (32 blocks), local from wtloc_sb (4)
                    for j in range(32):
                        gb = psA.tile([128, 128], f32, tag="gps",
                                      name=f"gnb{j}")
                        for kd in range(8):
                            sl = wt_sb[:, kd * C + 128 * j:
                                       kd * C + 128 * j + 128]
                            nc.tensor.matmul(gb[:], sl, sl,
                                             start=(kd == 0), stop=(kd == 7))
                        db = gscr.tile([128, 128], f32, tag="db")
                        nc.vector.tensor_tensor(db[:], gb[:], ident[:],
                                                op=AL.mult)
                        nc.vector.reduce_sum(
                            small[:, NSQ32 + j:NSQ32 + j + 1], db[:],
                            axis=mybir.AxisListType.X)
                    for m in range(NG):
                        gb = psA.tile([128, 128], f32, tag="gps",
                                      name=f"lnb{m}")
                        for kd in range(8):
                            sl = wtloc_sb[:, kd * CHUNK + 128 * m:
                                          kd * CHUNK + 128 * m + 128]
                            nc.tensor.matmul(gb[:], sl, sl,
                                             start=(kd == 0), stop=(kd == 7))
                        db = gscr.tile([128, 128], f32, tag="db")
                        nc.vector.tensor_tensor(db[:], gb[:], ident[:],
                                                op=AL.mult)
                        nc.vector.reduce_sum(
                            small[:, NSQL + m:NSQL + m + 1], db[:],
                            axis=mybir.AxisListType.X)

                    # inv = 1/max(sqrt(nsq), eps), cheap on row form
                    nc.scalar.activation(small[:, NSQ32:NSQ32 + 32],
                                         small[:, NSQ32:NSQ32 + 32], AF.Sqrt)
                    nc.vector.tensor_scalar_max(small[:, NSQ32:NSQ32 + 32],
                                                small[:, NSQ32:NSQ32 + 32],
                                                1e-8)
                    nc.vector.reciprocal(small[:, NSQ32:NSQ32 + 32],
                                         small[:, NSQ32:NSQ32 + 32])
                    nc.scalar.activation(small[:, INVL:INVL + 4],
                                         small[:, NSQL:NSQL + 4], AF.Sqrt)
                    nc.vector.tensor_scalar_max(small[:, INVL:INVL + 4],
                                                small[:, INVL:INVL + 4], 1e-8)
                    nc.vector.reciprocal(small[:, INVL:INVL + 4],
                                         small[:, INVL:INVL + 4])

                    # inv32 -> [32,128] -> DRAM [4096] -> bcast [128, C]
                    tp = psA.tile([32, 128], f32, tag="gps", name="tpp")
                    nc.tensor.transpose(tp[:], small[:, NSQ32:NSQ32 + 32],
                                        ident[:])
                    tps = gscr.tile([32, 128], f32, tag="tps")
                    nc.scalar.copy(tps[:], tp[:])
                    nrmd = dram_pool.tile([C], f32)
                    nc.sync.dma_start(
                        nrmd[:].rearrange("(j p) -> j p", p=128), tps[:])
                    nr = nrmd[:]
                    nc.gpsimd.dma_start(invb[:], bass.AP(
                        tensor=nr.tensor, offset=nr.offset,
                        ap=[[0, 128]] + [list(p) for p in nr.ap]))

                    # ---- Phase B: sim chunk -> conf (PE -> ACT direct) ----
                    if stage != "norm":
                        for m in range(NG):
                            tmph = [None, None]
                            for n in range(8):
                                g_ps = psA.tile([128, 512], f32, tag="gps")
                                for kd in range(8):
                                    nc.tensor.matmul(
                                        g_ps[:],
                                        wtloc_sb[:, kd * CHUNK + 128 * m:
                                                 kd * CHUNK + 128 * m + 128],
                                        wt_sb[:, kd * C + 512 * n:
                                              kd * C + 512 * n + 512],
                                        start=(kd == 0), stop=(kd == 7))
                                if exp_halves:
                                    h = n // 4
                                    if tmph[h] is None:
                                        tmph[h] = gscr.tile(
                                            [128, 2048], bf16, tag="gtmp",
                                            name=f"tmph{m}_{h}")
                                    nc.vector.tensor_tensor(
                                        tmph[h][:, 512 * (n % 4):
                                                512 * (n % 4) + 512],
                                        g_ps[:],
                                        invb[:, 512 * n:512 * n + 512],
                                        op=AL.mult)
                                    if n % 4 == 3:
                                        nc.scalar.activation(
                                            conf[:, m * C + 2048 * h:
                                                 m * C + 2048 * h + 2048],
                                            tmph[h][:], AF.Exp,
                                            scale=small[:, INVL + m:
                                                        INVL + m + 1],
                                            accum_out=small[
                                                :, ESG + m * 8 + h:
                                                ESG + m * 8 + h + 1])
                                    continue
                                tmp = gscr.tile([128, 512],
                                                bf16 if lse_in_b else f32,
                                                tag="gtmp")
                                nc.vector.tensor_tensor(
                                    tmp[:], g_ps[:],
                                    invb[:, 512 * n:512 * n + 512],
                                    op=AL.mult)
                                nc.scalar.activation(
                                    conf[:, m * C + 512 * n:
                                         m * C + 512 * n + 512],
                                    tmp[:], AF.Exp,
                                    scale=small[:, INVL + m:INVL + m + 1],
                                    accum_out=small[:, ESG + m * 8 + n:
                                                    ESG + m * 8 + n + 1])
                            # den = rowsum - e ; rden = 1/den
                            nred = 2 if exp_halves else 8
                            nc.vector.reduce_sum(
                                small[:, RDEN + m:RDEN + m + 1],
                                small[:, ESG + m * 8:ESG + m * 8 + nred],
                                axis=mybir.AxisListType.X)
                            nc.vector.tensor_scalar_add(
                                small[:, RDEN + m:RDEN + m + 1],
                                small[:, RDEN + m:RDEN + m + 1], -E_CONST)
                            nc.vector.reciprocal(
                                small[:, RDEN + m:RDEN + m + 1],
                                small[:, RDEN + m:RDEN + m + 1])
                            # conf = e * rden (diag kept; corrected later)
                            nc.vector.tensor_scalar_mul(
                                conf[:, m * C:(m + 1) * C],
                                conf[:, m * C:(m + 1) * C],
                                small[:, RDEN + m:RDEN + m + 1])
                            if lse_in_b:
                                # logsumexp stream interleaved with phase B:
                                # pred re-read so ACT overlaps the PE phase
                                for j in range(nkt):
                                    kt = m * nkt + j
                                    pt2 = lse_pool.tile(
                                        [128, C], bf16,
                                        name=f"ptl{kt}", tag="ptl")
                                    nc.sync.dma_start(
                                        pt2[:],
                                        predb[kt * 128:(kt + 1) * 128, :])
                                    nc.scalar.activation(
                                        pt2[:], pt2[:], AF.Exp,
                                        accum_out=esums[:, kt:kt + 1])

                # ============== Phase C: Q per group + lse ==============
                if stage not in ("norm", "g"):
                    with tc.tile_pool(name="psQ",
                                      bufs=(8 if q_split else 1),
                                      space="PSUM") as psQ:
                        for m in range(NG):
                            if q_split:
                                qs = [psQ.tile([128, 512], f32,
                                               name=f"q{m}_{n}", tag="qq")
                                      for n in range(8)]
                            else:
                                q_ps = psQ.tile([128, C], f32)
                            for ki in range(nkt):
                                kt = m * nkt + ki
                                pt = pred_pool.tile([128, C], bf16)
                                nc.sync.dma_start(
                                    pt[:], predb[kt * 128:(kt + 1) * 128, :])
                                oh = oh_pool.tile([128, 128], bf16)
                                oh_eng = (nc.gpsimd if oh_gpsimd
                                          else nc.vector)
                                oh_eng.tensor_scalar(
                                    oh[:], colidx[:], ltg_sb[:, kt:kt + 1],
                                    None, op0=AL.is_equal)
                                for n in range(8):
                                    qdst = (qs[n][:] if q_split else
                                            q_ps[:, 512 * n:512 * n + 512])
                                    nc.tensor.matmul(
                                        qdst, oh[:],
                                        pt[:, 512 * n:512 * n + 512],
                                        start=(ki == 0), stop=(ki == nkt - 1))
                                if not (early_lse or lse_in_b):
                                    es = escr_pool.tile([128, C], bf16,
                                                        tag="escr")
                                    nc.scalar.activation(
                                        es[:], pt[:], AF.Exp,
                                        accum_out=esums[:, kt:kt + 1])
                            # fold -0.9*pred_t and the conf-diagonal
                            # correction into conf_m: conf_m += s'*dmask_sh
                            # with s' = 9 - e*rden, then the single dot
                            # (coef -0.1) covers all Q terms.
                            nc.vector.tensor_scalar(
                                small[:, SPR + m:SPR + m + 1],
                                small[:, RDEN + m:RDEN + m + 1],
                                -E_CONST, 9.0, op0=AL.mult, op1=AL.add)
                            sdm = escr_pool.tile([128, C], bf16, tag="escr")
                            nc.vector.tensor_scalar(
                                sdm[:, 0:C - 128 * m],
                                dmask_sb[:, 0:C - 128 * m],
                                small[:, SPR + m:SPR + m + 1],
                                None, op0=AL.mult)
                            nc.vector.tensor_tensor(
                                conf[:, m * C + 128 * m:(m + 1) * C],
                                conf[:, m * C + 128 * m:(m + 1) * C],
                                sdm[:, 0:C - 128 * m], op=AL.add)
                            if q_split:
                                for n in range(8):
                                    scr2 = escr_pool.tile(
                                        [128, 512], bf16, tag="dchunk",
                                        name=f"dch{m}_{n}")
                                    nc.vector.tensor_tensor(
                                        scr2[:], qs[n][:],
                                        conf[:, m * C + 512 * n:
                                             m * C + 512 * n + 512],
                                        op=AL.mult)
                                    nc.vector.reduce_sum(
                                        small[:, ESG + m * 8 + n:
                                              ESG + m * 8 + n + 1],
                                        scr2[:], axis=mybir.AxisListType.X)
                                nc.vector.reduce_sum(
                                    small[:, FIN + TK + m:FIN + TK + m + 1],
                                    small[:, ESG + m * 8:ESG + m * 8 + 8],
                                    axis=mybir.AxisListType.X)
                            else:
                                scr2 = escr_pool.tile([128, C], bf16,
                                                      tag="escr")
                                nc.vector.tensor_tensor(
                                    scr2[:], q_ps[:],
                                    conf[:, m * C:(m + 1) * C],
                                    op=AL.mult)
                                nc.vector.reduce_sum(
                                    small[:, FIN + TK + m:FIN + TK + m + 1],
                                    scr2[:], axis=mybir.AxisListType.X)

            # ================= Phase D: final reduction =================
            with tc.tile_pool(name="psF", bufs=1, space="PSUM") as psF:
                outsb = escr_pool.tile([1, 1], f32, tag="outsb")
                if stage == "full":
                    nc.scalar.activation(small[:, LNV:LNV + TK], esums[:],
                                         AF.Ln)
                    nc.vector.tensor_tensor(small[:, FIN:FIN + TK],
                                            small[:, LNV:LNV + TK],
                                            vmask_sb[:], op=AL.mult)
                    nc.vector.memset(small[:, COEF:COEF + TK], 1.0)
                    nc.vector.memset(small[:, COEF + TK:COEF + TK + 4],
                                     -SMOOTHING)
                    nc.vector.tensor_tensor(
                        small[:, PROD:PROD + F],
                        small[:, FIN:FIN + F],
                        small[:, COEF:COEF + F], op=AL.mult)
                    nc.vector.reduce_sum(small[:, OUTC:OUTC + 1],
                                         small[:, PROD:PROD + F],
                                         axis=mybir.AxisListType.X)
                    fps = psF.tile([1, 1], f32)
                    nc.tensor.matmul(fps[:], small[:, OUTC:OUTC + 1],
                                     small[:, ONES:ONES + 1])
                    nc.scalar.copy(outsb[:], fps[:])
                else:
                    nc.vector.memset(outsb[:], 0.0)
                nc.sync.dma_start(out, outsb[:])

    nc.compile()
    if split_drains:
        _split_multiwait_drains(nc)
    return nc


def _prep(pred, weight, target):
    """Host-side sharding/staging. Returns (in_maps, nkt)."""
    pred = np.asarray(pred)
    weight = np.asarray(weight, dtype=np.float32)
    target = np.asarray(target).astype(np.int64)

    w_bf = weight.astype(ml_dtypes.bfloat16)
    wt_bf = np.ascontiguousarray(w_bf.T)

    core_of = (target // CHUNK).astype(np.int64)
    rows_per_core = [np.nonzero(core_of == k)[0] for k in range(NCORES)]

    # group sizes -> uniform tiles per group
    maxg = 1
    groups = []
    for k in range(NCORES):
        lt = target[rows_per_core[k]] - CHUNK * k
        gs = [rows_per_core[k][lt // 128 == m] for m in range(NG)]
        groups.append(gs)
        for g in gs:
            maxg = max(maxg, len(g))
    nkt = (maxg + 127) // 128
    TK = NG * nkt
    S = TK * 128

    pred_bf = pred.astype(ml_dtypes.bfloat16)

    in_maps = []
    for k in range(NCORES):
        predb = np.zeros((S, C), dtype=ml_dtypes.bfloat16)
        ltg = np.full((128, TK), 9999.0, dtype=np.float32)
        vm = np.zeros((128, TK), dtype=np.float32)
        for m in range(NG):
            idx = groups[k][m]
            off = m * nkt * 128
            n = len(idx)
            predb[off:off + n] = pred_bf[idx]
            r = off + np.arange(n)
            ltg[r & 127, r >> 7] = (target[idx] - CHUNK * k - 128 * m)
            vm[r & 127, r >> 7] = 1.0
        dmask = np.zeros((128, C), dtype=ml_dtypes.bfloat16)
        dmask[np.arange(128), CHUNK * k + np.arange(128)] = 1.0
        in_maps.append({
            "predb": predb,
            "wt_all": wt_bf,
            "wt_loc": np.ascontiguousarray(wt_bf[:, CHUNK * k:CHUNK * (k + 1)]),
            "ltg": ltg,
            "vmask": vm,
            "dmask": dmask,
        })
    return in_maps, nkt


def _install_trace_shims():
    """Make trace=True work in containers whose antenv lacks axon_hooks."""
    import sys
    import types
    try:
        import antenv.axon_hooks  # noqa: F401
    except ImportError:
        import antenv
        from trn_agent_boot.trn_boot import _ntff_profile_via_ctypes
        mod = types.ModuleType("antenv.axon_hooks")
        hook = _ntff_profile_via_ctypes("/opt/axon/libaxon_pjrt.so")
        mod.get_axon_ntff_profile_hook = lambda: hook
        mod.set_axon_ntff_profile_hook = lambda h: None
        sys.modules["antenv.axon_hooks"] = mod
        antenv.axon_hooks = mod
    import concourse.bass_utils as bu
    bu.upload_artifacts = lambda tmpdir: "local://" + tmpdir


def kernel(pred, weight, target):
    from concourse.bass_utils import run_bass_kernel_spmd
    global LAST_RESULTS

    in_maps, nkt = _prep(pred, weight, target)
    if nkt not in _cache:
        _cache[nkt] = _build(nkt)
    nc = _cache[nkt]

    trace = bool(int(os.environ.get("AKL_TRACE", "0")))
    if trace:
        _install_trace_shims()
    res = run_bass_kernel_spmd(nc, in_maps, core_ids=list(range(NCORES)),
                               trace=trace)
    LAST_RESULTS = res
    total = np.float64(0.0)
    for k in range(NCORES):
        total += np.float64(res.results[k]["out"][0, 0])
    return np.float32(total / B)



# revision 6
# speedup vs baseline: 2.0040x; 2.0040x over previous
"""AdaptiveLabelLoss Trainium2 kernel (8 NeuronCores, class-sharded).

loss = mean_b [ lse_b - 0.9*pred[b,t_b] - 0.1*conf[t_b].pred_b ]
where conf is the row-normalized exp cosine-similarity confusion matrix
(diagonal zeroed) and lse is logsumexp over pred rows. The Dirichlet
sample of the reference is replaced by its analytic mean (= conf row),
which matches the fixed-key sample mean to ~2e-5 relative.

Sharding: classes are partitioned into 32 bins (8 cores x 4 groups x
128 classes) balanced by target count, so each (core, group) bucket
holds ~512 rows and padding is ~zero. Rows are routed to the core/group
owning their target class. Weight rows are L2-normalized and scaled on
the host, then everything heavy runs in fp8e4 with DoubleRow matmuls:
  - Gram chunk sim[512, C] via W^T-sliced fp8 DoubleRow matmuls
  - exp(sim/S^2) straight out of PSUM on ACT (accum -> row sums)
  - Q = one-hot^T @ pred via fp8 DoubleRow (one-hots staged from host)
  - dot <conf, Q> via fused tensor_tensor_reduce on DVE
The -0.9*pred_t term and the conf-diagonal correction both reduce to
host-staged per-slot sums S[p,m] of own-target logits (the diagonal of
exp(sim) is e to ~0.5%, so subtracting e*S removes its contribution).
"""

import os
import numpy as np
import ml_dtypes

B, C, D = 16384, 4096, 1024
NCORES = 8
CHUNK = C // NCORES          # 512 classes per core
NG = 4                       # groups of 128 classes
KD = D // 128                # 8 contraction slices
SCALE = 16.0                 # host scale on normalized weight rows
ISCL2 = 1.0 / (SCALE * SCALE)
CONFIDENCE = 0.9
SMOOTHING = 0.1
E_CONST = float(np.exp(np.float32(1.0)))

_cache = {}
LAST_RESULTS = None  # for test harness introspection


def _split_multiwait_drains(nc, max_waits: int = 1):
    """Walrus (CoreV3) rejects instructions carrying many sem waits. The
    Tile kernel-tail drain waits on every engine/queue sem at once; split
    the extras onto preceding single-wait drains on the same engine."""
    import concourse.mybir as mybir
    import bass_rust
    for f in nc.m.functions:
        for bb in f.blocks:
            i = 0
            insts = bb.instructions
            while i < len(insts):
                inst = insts[i]
                si = inst.sync_info
                if si is not None and si.on_wait and len(si.on_wait) > max_waits:
                    waits = list(si.on_wait)
                    keep = waits[:max_waits]
                    extra = waits[max_waits:]
                    pre = []
                    for j, w in enumerate(extra):
                        d = mybir.InstDrain(
                            name=f"{inst.name}-sw{j}", ins=[], outs=[])
                        d.engine = inst.engine
                        d.sync_info = bass_rust.SyncInfo(
                            on_wait=[w], on_update=[])
                        pre.append(d)
                    inst.sync_info = bass_rust.SyncInfo(
                        on_wait=keep, on_update=list(si.on_update or []))
                    for j, d in enumerate(pre):
                        insts.insert(i + j, d)
                    i += len(pre)
                i += 1


def _build(nkt: int, stage: str = "full", split_drains: bool = True,
           use_dr: bool = True, pre0: int = 2):
    """Build + compile the SPMD program. nkt = 128-row tiles per group
    (even, for DoubleRow pairs). stage: debug knob - "in", "g", "full"."""
    import concourse.bass as bass
    import concourse.bacc as bacc
    import concourse.tile as tile
    import concourse.mybir as mybir
    import contextlib

    f32 = mybir.dt.float32
    bf16 = mybir.dt.bfloat16
    f8 = mybir.dt.float8e4
    AL = mybir.AluOpType
    AF = mybir.ActivationFunctionType
    DR = mybir.MatmulPerfMode.DoubleRow if use_dr else None

    assert nkt % 2 == 0
    TK = NG * nkt            # total row tiles
    NP = TK // 2             # pair tiles
    PPG = nkt // 2           # pairs per group

    nc = bacc.Bacc("TRN2", target_bir_lowering=False, debug=False,
                   num_devices=NCORES)

    predb = nc.dram_tensor("predb", [128, NP * 2 * C], f8,
                           kind="ExternalInput").ap()
    wta = nc.dram_tensor("wta", [128, KD, C], f8, kind="ExternalInput").ap()
    wtl = nc.dram_tensor("wtl", [128, KD * CHUNK], f8,
                         kind="ExternalInput").ap()
    ohh = nc.dram_tensor("ohh", [128, NP * 2 * 128], f8,
                         kind="ExternalInput").ap()
    meta = nc.dram_tensor("meta", [128, TK + NG], f32,
                          kind="ExternalInput").ap()
    out = nc.dram_tensor("out", [1, 1], f32, kind="ExternalOutput").ap()

    # small f32 scratch column map
    ESG = 0                    # [0, 16)   conf-exp partial row sums (m*4+q)
    RDEN = 16                  # [16, 20)  1/(rowsum - e)
    DOTG = 20                  # [20, 28)  half-group dots (h*4+m)
    DOTP = 28                  # [28, 32)  per-group dots
    TERM = 32                  # [32, 36)  dot - e*S
    F0 = 40                    # [40, 40+TK+8) final row
    ONES = F0 + TK + 8
    OUTC = ONES + 1
    NSMALL = OUTC + 2

    with tile.TileContext(nc) as tc:
        stack = contextlib.ExitStack()
        with stack:
            persist = stack.enter_context(tc.tile_pool(name="persist",
                                                       bufs=1))
            scre_pool = stack.enter_context(tc.tile_pool(name="scre",
                                                         bufs=2))
            scrt_pool = stack.enter_context(tc.tile_pool(name="scrt",
                                                         bufs=2))

            # ---- persistent tiles ----
            wt_sb = persist.tile([128, KD, C], f8)        # 32KB/part
            wtloc_sb = persist.tile([128, KD, CHUNK], f8)  # 4KB
            pred_sb = persist.tile([128, NP * 2 * C], f8)  # 8KB * NP
            oh_sb = persist.tile([128, 2 * NP, 128], f8)  # 2KB
            conf = persist.tile([128, NG * C], bf16)      # 32KB
            meta_sb = persist.tile([128, TK + NG], f32)
            esums = persist.tile([128, TK], f32)
            small = persist.tile([128, NSMALL], f32)

            # ---- input DMAs (ACT food first, then weights) ----
            nc.sync.dma_start(meta_sb[:], meta)
            nc.sync.dma_start(pred_sb[:, 0:2 * C], predb[:, 0:2 * C])
            nc.sync.dma_start(pred_sb[:, 2 * C:4 * C], predb[:, 2 * C:4 * C])
            nc.sync.dma_start(wtloc_sb[:], wtl)
            for n in range(8):
                nc.sync.dma_start(wt_sb[:, :, 512 * n:512 * (n + 1)],
                                  wta[:, :, 512 * n:512 * (n + 1)])
            nc.sync.dma_start(oh_sb[:], ohh)
            for u in range(2, NP):
                nc.sync.dma_start(pred_sb[:, 2 * C * u:2 * C * (u + 1)],
                                  predb[:, 2 * C * u:2 * C * (u + 1)])

            nc.vector.memset(small[:, ONES:ONES + 1], 1.0)

            # pred-exp job list: 2 per pair tile (j = 0, 1)
            pred_jobs = [(u, j) for u in range(NP) for j in range(2)]
            cursor = [0]

            def emit_pred_exp(njobs):
                for _ in range(njobs):
                    if cursor[0] >= len(pred_jobs):
                        return
                    u, j = pred_jobs[cursor[0]]
                    cursor[0] += 1
                    kt = 2 * u + j
                    scr = scre_pool.tile([128, C], bf16, tag="scre")
                    nc.scalar.activation(
                        scr[:], pred_sb[:, 2 * C * u + C * j:
                                        2 * C * u + C * (j + 1)], AF.Exp,
                        accum_out=esums[:, kt:kt + 1])

            if stage != "in":
                # ============== Phase A: Gram -> conf ==============
                # q-outer so the first matmuls only need wt chunks 2q,2q+1
                with tc.tile_pool(name="psA", bufs=4, space="PSUM") as psA:
                    emit_pred_exp(pre0)
                    for q in range(4):
                        g2 = [psA.tile([128, 1024], f32, tag="gps",
                                       name=f"g{m}_{q}") for m in range(NG)]
                        for m in range(NG):
                            for kdp in range(KD // 2):
                                for nn in range(2):
                                    n = 2 * q + nn
                                    nc.tensor.matmul(
                                        g2[m][:, 512 * nn:512 * nn + 512],
                                        wtloc_sb[:, 2 * kdp:2 * kdp + 2,
                                                 128 * m:128 * m + 128],
                                        wt_sb[:, 2 * kdp:2 * kdp + 2,
                                              512 * n:512 * n + 512],
                                        start=(kdp == 0), stop=(kdp == 3),
                                        perf_mode=DR)
                        for m in range(NG):
                            nc.scalar.activation(
                                conf[:, m * C + 1024 * q:
                                     m * C + 1024 * q + 1024],
                                g2[m][:], AF.Exp, scale=ISCL2,
                                accum_out=small[:, ESG + 4 * m + q:
                                                ESG + 4 * m + q + 1])
                        emit_pred_exp(1)
                    # rden = 1/(rowsum - e)
                    for m in range(NG):
                        nc.vector.reduce_sum(
                            small[:, RDEN + m:RDEN + m + 1],
                            small[:, ESG + 4 * m:ESG + 4 * m + 4],
                            axis=mybir.AxisListType.X)
                    nc.vector.tensor_scalar_add(
                        small[:, RDEN:RDEN + 4],
                        small[:, RDEN:RDEN + 4], -E_CONST)
                    nc.vector.reciprocal(
                        small[:, RDEN:RDEN + 4],
                        small[:, RDEN:RDEN + 4])

                # ============== Phase B: Q + dots ==============
                if stage not in ("g",):
                    with tc.tile_pool(name="psQ", bufs=2,
                                      space="PSUM") as psQ:
                        for m in range(NG):
                            for h in range(2):
                                qh = psQ.tile([128, 2048], f32, tag="qq",
                                              name=f"q{m}_{h}")
                                for ui in range(PPG):
                                    u = m * PPG + ui
                                    for nn in range(4):
                                        n = 4 * h + nn
                                        base = pred_sb[
                                            :, 2 * C * u + 512 * n:
                                            2 * C * u + 512 * n + 512]
                                        rhs3 = bass.AP(
                                            tensor=base.tensor,
                                            offset=base.offset,
                                            ap=[list(base.ap[0]),
                                                [C, 2], [1, 512]])
                                        nc.tensor.matmul(
                                            qh[:, 512 * nn:512 * nn + 512],
                                            oh_sb[:, 2 * u:2 * u + 2, :],
                                            rhs3,
                                            start=(ui == 0),
                                            stop=(ui == PPG - 1),
                                            perf_mode=DR)
                                if stage != "qmm":
                                    # dotg = sum(rden * Q * conf) per part
                                    scr = scrt_pool.tile([128, 2048], bf16,
                                                         tag="scrt")
                                    nc.vector.affine_mul_reduce(
                                        scr[:],
                                        small[:, DOTG + 4 * h + m:
                                              DOTG + 4 * h + m + 1],
                                        qh[:],
                                        conf[:, m * C + 2048 * h:
                                             m * C + 2048 * h + 2048],
                                        small[:, RDEN + m:RDEN + m + 1],
                                        0.0)
                                emit_pred_exp(1)

            emit_pred_exp(99)

            # ================= Phase C: final reduction =================
            with tc.tile_pool(name="psF", bufs=1, space="PSUM") as psF:
                outsb = scre_pool.tile([1, 1], f32, tag="outsb")
                if stage == "full":
                    # dotp = half sums (already rden-scaled by AMR)
                    nc.vector.tensor_tensor(
                        small[:, DOTP:DOTP + 4],
                        small[:, DOTG:DOTG + 4],
                        small[:, DOTG + 4:DOTG + 8], op=AL.add)
                    # term = dotp - e * rden * S
                    nc.vector.tensor_tensor(
                        small[:, TERM:TERM + 4],
                        meta_sb[:, TK:TK + NG],
                        small[:, RDEN:RDEN + 4], op=AL.mult)
                    nc.vector.tensor_scalar(
                        small[:, TERM:TERM + 4],
                        small[:, TERM:TERM + 4], -E_CONST, None,
                        op0=AL.mult)
                    nc.vector.tensor_tensor(
                        small[:, TERM:TERM + 4], small[:, TERM:TERM + 4],
                        small[:, DOTP:DOTP + 4], op=AL.add)
                    # f = -0.1 * term
                    nc.vector.tensor_scalar(
                        small[:, F0 + TK:F0 + TK + 4],
                        small[:, TERM:TERM + 4],
                        -SMOOTHING, None, op0=AL.mult)
                    # -0.9 * S
                    nc.vector.tensor_scalar(
                        small[:, F0 + TK + 4:F0 + TK + 8],
                        meta_sb[:, TK:TK + NG], -CONFIDENCE, None,
                        op0=AL.mult)
                    # lse: ln(esums), masked by vmask
                    nc.scalar.activation(small[:, F0:F0 + TK], esums[:],
                                         AF.Ln)
                    nc.vector.tensor_tensor(
                        small[:, F0:F0 + TK], small[:, F0:F0 + TK],
                        meta_sb[:, 0:TK], op=AL.mult)
                    nc.vector.reduce_sum(small[:, OUTC:OUTC + 1],
                                         small[:, F0:F0 + TK + 8],
                                         axis=mybir.AxisListType.X)
                    fps = psF.tile([1, 1], f32)
                    nc.tensor.matmul(fps[:], small[:, OUTC:OUTC + 1],
                                     small[:, ONES:ONES + 1])
                    nc.scalar.copy(outsb[:], fps[:])
                else:
                    nc.vector.memset(outsb[:], 0.0)
                nc.sync.dma_start(out, outsb[:])

    nc.compile()
    if split_drains:
        _split_multiwait_drains(nc)
    return nc


def _pack_classes(counts):
    """Partition C classes into 32 bins (8 cores x 4 groups), each with
    exactly 128 classes, balancing row counts (LPT + pairwise repair).
    Returns (bins: list of 32 int64 arrays, cap: max bin row count)."""
    NB = NCORES * NG
    PER = C // NB
    order = np.argsort(-counts, kind="stable")
    bins = [[] for _ in range(NB)]
    sums = np.zeros(NB, dtype=np.int64)
    ncls = np.zeros(NB, dtype=np.int64)
    for c in order:
        avail = np.nonzero(ncls < PER)[0]
        b = avail[np.argmin(sums[avail])]
        bins[b].append(int(c))
        sums[b] += counts[c]
        ncls[b] += 1
    cap = int(np.ceil(counts.sum() / NB))
    for _ in range(4 * C):
        hi = int(np.argmax(sums))
        if sums[hi] <= cap:
            break
        lo = int(np.argmin(sums))
        need = sums[hi] - cap
        ch, cl = bins[hi], bins[lo]
        clc = counts[cl]
        best = None
        for i, c1 in enumerate(ch):
            d1 = counts[c1]
            if d1 == 0:
                continue
            j = int(np.argmin(np.abs(clc - (d1 - need))))
            d = d1 - clc[j]
            if d > 0 and (best is None or
                          abs(d - need) < abs(best[0] - need)):
                best = (d, i, j)
        if best is None:
            break
        d, i, j = best
        ch[i], cl[j] = cl[j], ch[i]
        sums[hi] -= d
        sums[lo] += d
    return [np.array(b, dtype=np.int64) for b in bins], int(sums.max())


def _prep(pred, weight, target):
    """Host-side sharding/staging. Returns (in_maps, nkt)."""
    pred = np.asarray(pred, dtype=np.float32)
    weight = np.asarray(weight, dtype=np.float32)
    target = np.asarray(target).astype(np.int64)

    counts = np.bincount(target, minlength=C)
    bins, cap = _pack_classes(counts)
    nkt = (cap + 127) // 128
    nkt += nkt % 2                       # even, for DoubleRow pairs
    TK = NG * nkt
    NP = TK // 2

    # normalized, scaled fp8 weight (rows of W)
    norms = np.maximum(np.sqrt((weight.astype(np.float64) ** 2)
                               .sum(axis=1)), 1e-8)
    wn = (weight / norms[:, None].astype(np.float32)) * SCALE
    wn8 = wn.astype(ml_dtypes.float8_e4m3)          # [C, D]
    wnT = np.ascontiguousarray(wn8.T)               # [D, C]
    # [128, KD, C] layout: [p, j, c] = wnT[j*128+p, c]
    wta_host = np.ascontiguousarray(
        wnT.reshape(KD, 128, C).transpose(1, 0, 2))

    pred8 = pred.astype(ml_dtypes.float8_e4m3)
    rows_by_class = [np.nonzero(target == c)[0] for c in range(C)]

    in_maps = []
    for k in range(NCORES):
        cls = [bins[NG * k + m] for m in range(NG)]
        cols = np.concatenate(cls)                  # [512]
        wl = wnT[:, cols]                           # [D, 512]
        wtl_host = np.ascontiguousarray(
            wl.reshape(KD, 128, CHUNK).transpose(1, 0, 2)
            .reshape(128, KD * CHUNK))

        predb = np.zeros((128, NP * 2 * C), dtype=ml_dtypes.float8_e4m3)
        ohh = np.zeros((128, NP * 2 * 128), dtype=ml_dtypes.float8_e4m3)
        vmask = np.zeros((128, TK), dtype=np.float32)
        S = np.zeros((128, NG), dtype=np.float64)
        for m in range(NG):
            rows = []
            slots = []
            for s, c in enumerate(cls[m]):
                r = rows_by_class[c]
                rows.append(r)
                slots.append(np.full(len(r), s, dtype=np.int64))
            rows = np.concatenate(rows)
            slots = np.concatenate(slots)
            nrow = len(rows)
            assert nrow <= nkt * 128
            gpred = pred8[rows]                     # [nrow, C]
            gt = pred[rows, target[rows]].astype(np.float64)
            for ktl in range((nrow + 127) // 128):
                sel = slice(128 * ktl, min(128 * (ktl + 1), nrow))
                cnt = sel.stop - sel.start
                ktg = m * nkt + ktl
                colbase = ktg * C        # (u*2+j)*C == ktg*C
                predb[0:cnt, colbase:colbase + C] = gpred[sel]
                ohh[np.arange(cnt), ktg * 128 + slots[sel]] = 1.0
                vmask[0:cnt, ktg] = 1.0
                np.add.at(S, (np.arange(cnt), np.full(cnt, m)), gt[sel])
        meta_host = np.concatenate([vmask, S.astype(np.float32)], axis=1)

        in_maps.append({
            "predb": predb,
            "wta": wta_host,
            "wtl": wtl_host,
            "ohh": ohh,
            "meta": meta_host,
        })
    return in_maps, nkt


def _install_trace_shims():
    """Make trace=True work in containers whose antenv lacks axon_hooks."""
    import sys
    import types
    try:
        import antenv.axon_hooks  # noqa: F401
    except ImportError:
        import antenv
        from trn_agent_boot.trn_boot import _ntff_profile_via_ctypes
        mod = types.ModuleType("antenv.axon_hooks")
        hook = _ntff_profile_via_ctypes("/opt/axon/libaxon_pjrt.so")
        mod.get_axon_ntff_profile_hook = lambda: hook
        mod.set_axon_ntff_profile_hook = lambda h: None
        sys.modules["antenv.axon_hooks"] = mod
        antenv.axon_hooks = mod
    import concourse.bass_utils as bu
    bu.upload_artifacts = lambda tmpdir: "local://" + tmpdir


def kernel(pred, weight, target):
    from concourse.bass_utils import run_bass_kernel_spmd
    global LAST_RESULTS

    in_maps, nkt = _prep(pred, weight, target)
    if nkt not in _cache:
        _cache[nkt] = _build(nkt)
    nc = _cache[nkt]

    trace = bool(int(os.environ.get("AKL_TRACE", "0")))
    if trace:
        _install_trace_shims()
    res = run_bass_kernel_spmd(nc, in_maps, core_ids=list(range(NCORES)),
                               trace=trace)
    LAST_RESULTS = res
    total = np.float64(0.0)
    for k in range(NCORES):
        total += np.float64(res.results[k]["out"][0, 0])
    return np.float32(total / B)
